# revision 1
# baseline (speedup 1.0000x reference)
"""Trainium2 Bass kernel for nn_BOREP (dense_mlp):

    out[s, b, o] = einsum('sbi,oi->sbo', x, W) + bias[o]
    x [256, 64, 1024] f32, W [4096, 1024] f32, bias [4096] f32 -> out [256, 64, 4096] f32

Strategy
--------
Data-parallel over 8 NeuronCores: shard x along seq (axis 0), 32 rows/core,
i.e. per-core A = x-shard reshaped to [2048, 1024]; W and bias replicated.
Per core: out_shard = A @ W.T + bias -> [2048, 4096].

The per-core matmul runs as an fp16 triple-split ("f16x3"): each fp32 operand
is split into hi = fp16(v) and lo = fp16(v - hi) (fp16 keeps 11+11 significand
bits; TRN2's PE handles fp16 subnormals exactly, verified on HW), and

    A @ W.T  ~=  Ah@Wh.T + Ah@Wl.T + Al@Wh.T     (the Al@Wl.T term is ~2^-22)

All three products accumulate into the same PSUM bank in fp32. fp16 matmuls
stream at 1 cycle/row on the PE vs 4 cycles/row for fp32, so this runs at
3 cycles/row -- 1.6x faster than native fp32 with ~4e-7 relative error
(fp32-class; HW-measured 4.2e-7 vs a float64 reference).

Layout: host pre-blocks operands so every DMA lands [128, kt, free] tiles with
>=2KB contiguous runs per partition. Contraction dim k lives on partitions;
W stays SBUF-resident (16 MB as fp16 hi+lo); x m-slices are double-buffered;
each [128m, 512n] output tile is one PSUM bank, bias is added during the
PSUM->SBUF copy (DVE tensor_tensor), 8-bank PSUM pipelining.
"""
import sys

if "/opt/trn_rl_repo" not in sys.path:
    sys.path.insert(0, "/opt/trn_rl_repo")

import numpy as np

# Problem constants (hardcoded per contest contract)
SEQ, BATCH, IN_DIM, OUT_DIM = 256, 64, 1024, 4096
N_CORES = 8
P = 128
K = IN_DIM
M = SEQ * BATCH // N_CORES     # 2048 rows per core
N = OUT_DIM
KT = K // P                    # 8 k-tiles
TM = 128                       # out-tile rows (PSUM partitions)
TN = 512                       # out-tile cols (one PSUM bank of fp32)
MT = M // TM                   # 16
NT = N // TN                   # 8

_cache = {}


def _build_nc():
    import concourse.mybir as mybir
    import concourse.tile as tile
    from concourse import bacc
    from contextlib import ExitStack

    F32 = mybir.dt.float32
    F16 = mybir.dt.float16

    nc = bacc.Bacc("TRN2", target_bir_lowering=False, debug=False)

    xh = nc.dram_tensor("xh", [MT, P, KT, TM], F16, kind="ExternalInput").ap()
    xl = nc.dram_tensor("xl", [MT, P, KT, TM], F16, kind="ExternalInput").ap()
    wh = nc.dram_tensor("wh", [NT, P, KT, TN], F16, kind="ExternalInput").ap()
    wl = nc.dram_tensor("wl", [NT, P, KT, TN], F16, kind="ExternalInput").ap()
    bias = nc.dram_tensor("bias", [P, N], F32, kind="ExternalInput").ap()
    out = nc.dram_tensor("out", [M, N], F32, kind="ExternalOutput").ap()

    with tile.TileContext(nc) as tc:
        with ExitStack() as ctx:
            wpool = ctx.enter_context(tc.tile_pool(name="wpool", bufs=1))
            xpool = ctx.enter_context(tc.tile_pool(name="xpool", bufs=2))
            opool = ctx.enter_context(tc.tile_pool(name="opool", bufs=4))
            cpool = ctx.enter_context(tc.tile_pool(name="cpool", bufs=1))
            ps = ctx.enter_context(tc.tile_pool(name="ps", bufs=8, space="PSUM"))

            bias_sb = cpool.tile([P, N], F32)
            nc.sync.dma_start(bias_sb[:], bias[:])

            # W resident, emitted n-slice-major so compute starts after the
            # first slice lands while the rest stream in behind it.
            w_sb = []
            for n in range(NT):
                th = wpool.tile([P, KT, TN], F16, tag=f"wh_{n}")
                nc.sync.dma_start(th[:], wh[n])
                tl = wpool.tile([P, KT, TN], F16, tag=f"wl_{n}")
                nc.sync.dma_start(tl[:], wl[n])
                w_sb.append((th, tl))

            for m in range(MT):
                xh_sb = xpool.tile([P, KT, TM], F16, tag="xh")
                nc.sync.dma_start(xh_sb[:], xh[m])
                xl_sb = xpool.tile([P, KT, TM], F16, tag="xl")
                nc.sync.dma_start(xl_sb[:], xl[m])

                for n in range(NT):
                    psum = ps.tile([P, TN], F32)
                    groups = [(xh_sb, w_sb[n][0]),
                              (xh_sb, w_sb[n][1]),
                              (xl_sb, w_sb[n][0])]
                    n_mm = len(groups) * KT
                    i = 0
                    for (xt, wt) in groups:
                        for k in range(KT):
                            nc.tensor.matmul(
                                psum[:], xt[:, k], wt[:, k],
                                start=(i == 0), stop=(i == n_mm - 1),
                            )
                            i += 1
                    o_sb = opool.tile([P, TN], F32)
                    nc.vector.tensor_tensor(
                        o_sb[:], psum[:], bias_sb[:, n * TN:(n + 1) * TN],
                        mybir.AluOpType.add,
                    )
                    nc.sync.dma_start(
                        out[m * TM:(m + 1) * TM, n * TN:(n + 1) * TN], o_sb[:]
                    )
    nc.compile()
    return nc


def get_nc():
    if "nc" not in _cache:
        _cache["nc"] = _build_nc()
    return _cache["nc"]


def _blk_x(a2d):
    """[M, K] f16 -> [MT, P, KT, TM] with blk[m, p, k, j] = a2d[m*TM+j, k*P+p]."""
    aT = np.ascontiguousarray(a2d.T)  # [K, M]
    return np.ascontiguousarray(aT.reshape(KT, P, MT, TM).transpose(2, 1, 0, 3))


def _blk_w(wt):
    """[K, N] f16 -> [NT, P, KT, TN] with blk[n, p, k, j] = wt[k*P+p, n*TN+j]."""
    return np.ascontiguousarray(wt.reshape(KT, P, NT, TN).transpose(2, 1, 0, 3))


def prep_in_maps(x, W, b):
    x = np.asarray(x, dtype=np.float32)
    W = np.asarray(W, dtype=np.float32)
    b = np.asarray(b, dtype=np.float32)

    A = x.reshape(SEQ * BATCH, K)
    wh = W.astype(np.float16)
    wl = (W - wh.astype(np.float32)).astype(np.float16)
    whb = _blk_w(np.ascontiguousarray(wh.T))
    wlb = _blk_w(np.ascontiguousarray(wl.T))
    bias_bcast = np.ascontiguousarray(np.broadcast_to(b, (P, N)))

    in_maps = []
    for c in range(N_CORES):
        Ac = A[c * M:(c + 1) * M]
        ah = Ac.astype(np.float16)
        al = (Ac - ah.astype(np.float32)).astype(np.float16)
        in_maps.append({
            "xh": _blk_x(ah), "xl": _blk_x(al),
            "wh": whb, "wl": wlb, "bias": bias_bcast,
        })
    return in_maps


def kernel(x, W, b):
    from concourse.bass_utils import run_bass_kernel_spmd

    in_maps = prep_in_maps(x, W, b)
    nc = get_nc()
    res = run_bass_kernel_spmd(nc, in_maps, core_ids=list(range(N_CORES)))
    full = np.concatenate([r["out"] for r in res.results], axis=0)
    return full.reshape(SEQ, BATCH, OUT_DIM).astype(np.float32)


# revision 2
# speedup vs baseline: 1.0304x; 1.0304x over previous
"""Trainium2 Bass kernel for nn_BOREP (dense_mlp):

    out[s, b, o] = einsum('sbi,oi->sbo', x, W) + bias[o]
    x [256, 64, 1024] f32, W [4096, 1024] f32, bias [4096] f32 -> out [256, 64, 4096] f32

Strategy
--------
Data-parallel over 8 NeuronCores: shard x along seq (axis 0), 32 rows/core,
i.e. per-core A = x-shard reshaped to [2048, 1024]; W and bias replicated.
Per core: out_shard = A @ W.T + bias -> [2048, 4096].

The per-core matmul runs as an fp16 triple-split ("f16x3"): each fp32 operand
is split into hi = fp16(v) and lo = fp16(v - hi) (fp16 keeps 11+11 significand
bits; TRN2's PE handles fp16 subnormals exactly, verified on HW), and

    A @ W.T  ~=  Ah@Wh.T + Ah@Wl.T + Al@Wh.T     (the Al@Wl.T term is ~2^-22)

All three products accumulate into the same PSUM bank in fp32. fp16 matmuls
stream at 1 cycle/row on the PE vs 4 cycles/row for fp32, so this runs at
3 cycles/row -- 1.6x faster than native fp32 with ~4e-7 relative error
(fp32-class; HW-measured 4.2e-7 vs a float64 reference).

Layout: host pre-blocks operands so every DMA lands [128, kt, free] tiles with
>=2KB contiguous runs per partition. Contraction dim k lives on partitions;
W stays SBUF-resident (16 MB as fp16 hi+lo); x m-slices are double-buffered;
each [128m, 512n] output tile is one PSUM bank, bias is added during the
PSUM->SBUF copy (DVE tensor_tensor), 8-bank PSUM pipelining.
"""
import sys

if "/opt/trn_rl_repo" not in sys.path:
    sys.path.insert(0, "/opt/trn_rl_repo")

import numpy as np

# Problem constants (hardcoded per contest contract)
SEQ, BATCH, IN_DIM, OUT_DIM = 256, 64, 1024, 4096
N_CORES = 8
P = 128
K = IN_DIM
M = SEQ * BATCH // N_CORES     # 2048 rows per core
N = OUT_DIM
KT = K // P                    # 8 k-tiles
TM = 128                       # out-tile rows (PSUM partitions)
TN = 512                       # out-tile cols (one PSUM bank of fp32)
MT = M // TM                   # 16
NT = N // TN                   # 8

_cache = {}


def _build_nc(repeat: int = 1):
    import concourse.mybir as mybir
    import concourse.tile as tile
    from concourse import bacc
    from contextlib import ExitStack

    F32 = mybir.dt.float32
    F16 = mybir.dt.float16

    nc = bacc.Bacc("TRN2", target_bir_lowering=False, debug=False)

    xh = nc.dram_tensor("xh", [MT, P, KT, TM], F16, kind="ExternalInput").ap()
    xl = nc.dram_tensor("xl", [MT, P, KT, TM], F16, kind="ExternalInput").ap()
    wh = nc.dram_tensor("wh", [NT, P, KT, TN], F16, kind="ExternalInput").ap()
    wl = nc.dram_tensor("wl", [NT, P, KT, TN], F16, kind="ExternalInput").ap()
    bias = nc.dram_tensor("bias", [P, N], F32, kind="ExternalInput").ap()
    out = nc.dram_tensor("out", [M, N], F32, kind="ExternalOutput").ap()

    with tile.TileContext(nc) as tc:
        with ExitStack() as ctx:
            wpool = ctx.enter_context(tc.tile_pool(name="wpool", bufs=1))
            xpool = ctx.enter_context(tc.tile_pool(name="xpool", bufs=2))
            opool = ctx.enter_context(tc.tile_pool(name="opool", bufs=4))
            cpool = ctx.enter_context(tc.tile_pool(name="cpool", bufs=1))
            ps = ctx.enter_context(tc.tile_pool(name="ps", bufs=8, space="PSUM"))

            bias_sb = cpool.tile([P, N], F32)

            for _ in range(repeat):
                # Prologue ordering matters: the first m-slice of x is queued
                # before W so the PE's first accumulation group isn't gated on
                # the (larger) W transfers; bias is deferred behind W n=0 --
                # it is first consumed by the DVE, several microseconds later.
                xh0 = xpool.tile([P, KT, TM], F16, tag="xh")
                nc.sync.dma_start(xh0[:], xh[0])
                xl0 = xpool.tile([P, KT, TM], F16, tag="xl")
                nc.sync.dma_start(xl0[:], xl[0])

                # W resident, emitted n-slice-major so compute starts after
                # the first slice lands while the rest stream in behind it.
                w_sb = []
                for n in range(NT):
                    th = wpool.tile([P, KT, TN], F16, tag=f"wh_{n}")
                    nc.sync.dma_start(th[:], wh[n])
                    tl = wpool.tile([P, KT, TN], F16, tag=f"wl_{n}")
                    nc.sync.dma_start(tl[:], wl[n])
                    w_sb.append((th, tl))
                    if n == 0:
                        nc.sync.dma_start(bias_sb[:], bias[:])

                for m in range(MT):
                    if m == 0:
                        xh_sb, xl_sb = xh0, xl0
                    else:
                        xh_sb = xpool.tile([P, KT, TM], F16, tag="xh")
                        nc.sync.dma_start(xh_sb[:], xh[m])
                        xl_sb = xpool.tile([P, KT, TM], F16, tag="xl")
                        nc.sync.dma_start(xl_sb[:], xl[m])

                    for n in range(NT):
                        psum = ps.tile([P, TN], F32)
                        groups = [(xh_sb, w_sb[n][0]),
                                  (xh_sb, w_sb[n][1]),
                                  (xl_sb, w_sb[n][0])]
                        n_mm = len(groups) * KT
                        i = 0
                        for (xt, wt) in groups:
                            for k in range(KT):
                                nc.tensor.matmul(
                                    psum[:], xt[:, k], wt[:, k],
                                    start=(i == 0), stop=(i == n_mm - 1),
                                )
                                i += 1
                        o_sb = opool.tile([P, TN], F32)
                        nc.vector.tensor_tensor(
                            o_sb[:], psum[:], bias_sb[:, n * TN:(n + 1) * TN],
                            mybir.AluOpType.add,
                        )
                        nc.sync.dma_start(
                            out[m * TM:(m + 1) * TM, n * TN:(n + 1) * TN], o_sb[:]
                        )
    nc.compile()
    return nc


def get_nc():
    if "nc" not in _cache:
        _cache["nc"] = _build_nc()
    return _cache["nc"]


def _blk_x(a2d):
    """[M, K] f16 -> [MT, P, KT, TM] with blk[m, p, k, j] = a2d[m*TM+j, k*P+p]."""
    aT = np.ascontiguousarray(a2d.T)  # [K, M]
    return np.ascontiguousarray(aT.reshape(KT, P, MT, TM).transpose(2, 1, 0, 3))


def _blk_w(wt):
    """[K, N] f16 -> [NT, P, KT, TN] with blk[n, p, k, j] = wt[k*P+p, n*TN+j]."""
    return np.ascontiguousarray(wt.reshape(KT, P, NT, TN).transpose(2, 1, 0, 3))


def prep_in_maps(x, W, b):
    x = np.asarray(x, dtype=np.float32)
    W = np.asarray(W, dtype=np.float32)
    b = np.asarray(b, dtype=np.float32)

    A = x.reshape(SEQ * BATCH, K)
    wh = W.astype(np.float16)
    wl = (W - wh.astype(np.float32)).astype(np.float16)
    whb = _blk_w(np.ascontiguousarray(wh.T))
    wlb = _blk_w(np.ascontiguousarray(wl.T))
    bias_bcast = np.ascontiguousarray(np.broadcast_to(b, (P, N)))

    in_maps = []
    for c in range(N_CORES):
        Ac = A[c * M:(c + 1) * M]
        ah = Ac.astype(np.float16)
        al = (Ac - ah.astype(np.float32)).astype(np.float16)
        in_maps.append({
            "xh": _blk_x(ah), "xl": _blk_x(al),
            "wh": whb, "wl": wlb, "bias": bias_bcast,
        })
    return in_maps


def kernel(x, W, b):
    from concourse.bass_utils import run_bass_kernel_spmd

    in_maps = prep_in_maps(x, W, b)
    nc = get_nc()
    res = run_bass_kernel_spmd(nc, in_maps, core_ids=list(range(N_CORES)))
    full = np.concatenate([r["out"] for r in res.results], axis=0)
    return full.reshape(SEQ, BATCH, OUT_DIM).astype(np.float32)


# revision 3
# speedup vs baseline: 1.4098x; 1.3681x over previous
"""Trainium2 Bass kernel for nn_BOREP (dense_mlp):

    out[s, b, o] = einsum('sbi,oi->sbo', x, W) + bias[o]
    x [256, 64, 1024] f32, W [4096, 1024] f32, bias [4096] f32 -> out [256, 64, 4096] f32

Strategy
--------
Data-parallel over 8 NeuronCores: shard x along seq (axis 0), 32 timesteps per
core, i.e. per-core A = x-shard reshaped to [2048, 1024]; W and bias
replicated. Per core: out_shard = A @ W.T + bias -> [2048, 4096].

Per-core numeric scheme ("f32r main + fp8-DoubleRow correction"):
TRN2's PE runs fp32 matmul at 4 cycles/row, but the `float32r` dtype streams
at 1 cycle/row (free dim >= 256) while keeping exactly 12 significand bits
(HW-verified: 12-bit values pass through bit-exactly in both operand roles).
So the fp32 product is computed as an exact 12-bit main term plus a small
correction evaluated in fp8 at double rate:

    xh = rtn12(x), dx = x - xh   (|dx| <= 2^-12 |x|);  wh = rtn12(W), dw likewise
    A @ W.T  =  Ah @ Wh.T                   exact products of 12-bit values,
                                            float32r @ 1 cyc/row
             +  (dx @ W.T + A @ dw.T)       ~2^-12-scale correction, e4m3 fp8
                                            with DoubleRow perf mode (2 k-tiles
                                            per instruction, ~0.5 cyc/row)
             (+ dx @ dw.T ~ 2^-24, dropped)

The fp8 correction operands carry power-of-2 scales chosen so both cross
products land in one PSUM bank at a common 2^16 scale: dx8 = e4m3(dx * 2^12),
w8 = e4m3(W * 2^4), x8 = e4m3(x), dw8 = e4m3(dw * 2^16). The final output is
out = psum_main + 2^-16 * psum_cross + bias (DVE ops during PSUM->SBUF copy).
Total PE cost ~2.1 cycles/row vs 4 for native fp32. HW-measured: ~345 us/core
body (vs ~1100 us native fp32, ~630 us for an fp16 triple-split), max rel
error 4.3e-06 (absmax ~5.9e-05 on an output scale of 13.6), bit-deterministic.

Layout: host pre-blocks operands so every DMA lands [128, kt, free] tiles with
>=1KB-contiguous runs per partition; contraction dim k on SBUF partitions.
Loop is n-outer with the x-side SBUF-resident (~96KB/partition) and W streamed
once (24MB total traffic), double-buffered; each [128m, 512n] output tile uses
two PSUM banks (main + cross), 4-deep pipelining.
"""
import sys

if "/opt/trn_rl_repo" not in sys.path:
    sys.path.insert(0, "/opt/trn_rl_repo")

import numpy as np
import ml_dtypes

# Problem constants (hardcoded per contest contract)
SEQ, BATCH, IN_DIM, OUT_DIM = 256, 64, 1024, 4096
N_CORES = 8
P = 128
K = IN_DIM
M = SEQ * BATCH // N_CORES     # 2048 rows per core
N = OUT_DIM
KT = K // P                    # 8 k-tiles
TM = 128                       # out-tile rows (PSUM partitions)
TN = 512                       # out-tile cols (one PSUM bank of fp32)
MT = M // TM                   # 16
NT = N // TN                   # 8

E4M3 = ml_dtypes.float8_e4m3

_cache = {}


def _build_nc(repeat: int = 1):
    import concourse.mybir as mybir
    import concourse.tile as tile
    from concourse import bacc
    from contextlib import ExitStack

    F32 = mybir.dt.float32
    F32R = mybir.dt.float32r
    F8 = mybir.dt.float8e4

    nc = bacc.Bacc("TRN2", target_bir_lowering=False, debug=False)

    xh = nc.dram_tensor("xh", [MT, P, KT, TM], F32R, kind="ExternalInput").ap()
    dx8 = nc.dram_tensor("dx8", [MT, P, KT, TM], F8, kind="ExternalInput").ap()
    x8 = nc.dram_tensor("x8", [MT, P, KT, TM], F8, kind="ExternalInput").ap()
    wh = nc.dram_tensor("wh", [NT, P, KT, TN], F32R, kind="ExternalInput").ap()
    w8 = nc.dram_tensor("w8", [NT, P, KT, TN], F8, kind="ExternalInput").ap()
    dw8 = nc.dram_tensor("dw8", [NT, P, KT, TN], F8, kind="ExternalInput").ap()
    bias = nc.dram_tensor("bias", [P, N], F32, kind="ExternalInput").ap()
    out = nc.dram_tensor("out", [M, N], F32, kind="ExternalOutput").ap()

    with tile.TileContext(nc) as tc:
        with ExitStack() as ctx:
            xpool = ctx.enter_context(tc.tile_pool(name="xpool", bufs=1))
            wpool = ctx.enter_context(tc.tile_pool(name="wpool", bufs=2))
            opool = ctx.enter_context(tc.tile_pool(name="opool", bufs=4))
            cpool = ctx.enter_context(tc.tile_pool(name="cpool", bufs=1))
            ps = ctx.enter_context(tc.tile_pool(name="ps", bufs=4, space="PSUM"))

            bias_sb = cpool.tile([P, N], F32)

            for _ in range(repeat):
                # x-side resident; m=0 slices first so compute starts early,
                # bias deferred (first consumed by DVE, later).
                xh_sb, dx_sb, x8_sb = [], [], []
                for m in range(MT):
                    t1 = xpool.tile([P, KT, TM], F32R, tag=f"xh_{m}")
                    nc.sync.dma_start(t1[:], xh[m])
                    t2 = xpool.tile([P, KT, TM], F8, tag=f"dx_{m}")
                    nc.sync.dma_start(t2[:], dx8[m])
                    t3 = xpool.tile([P, KT, TM], F8, tag=f"x8_{m}")
                    nc.sync.dma_start(t3[:], x8[m])
                    xh_sb.append(t1); dx_sb.append(t2); x8_sb.append(t3)
                    if m == 0:
                        nc.sync.dma_start(bias_sb[:], bias[:])

                for n in range(NT):
                    wh_sb = wpool.tile([P, KT, TN], F32R, tag="wh")
                    nc.sync.dma_start(wh_sb[:], wh[n])
                    w8_sb = wpool.tile([P, KT, TN], F8, tag="w8")
                    nc.sync.dma_start(w8_sb[:], w8[n])
                    dw_sb = wpool.tile([P, KT, TN], F8, tag="dw")
                    nc.sync.dma_start(dw_sb[:], dw8[n])

                    for m in range(MT):
                        pm = ps.tile([P, TN], F32)
                        for k in range(KT):
                            nc.tensor.matmul(
                                pm[:], xh_sb[m][:, k], wh_sb[:, k],
                                start=(k == 0), stop=(k == KT - 1),
                            )
                        pc = ps.tile([P, TN], F32)
                        # DoubleRow: [P, KT, X] viewed as [P, KT//2, 2, X];
                        # each instruction contracts 2 k-tiles (256 values).
                        dxv = dx_sb[m].rearrange("p (j i) t -> p j i t", i=2)
                        x8v = x8_sb[m].rearrange("p (j i) t -> p j i t", i=2)
                        w8v = w8_sb.rearrange("p (j i) t -> p j i t", i=2)
                        dwv = dw_sb.rearrange("p (j i) t -> p j i t", i=2)
                        n_dr = KT
                        i = 0
                        for (lv, rv) in ((dxv, w8v), (x8v, dwv)):
                            for j in range(KT // 2):
                                nc.tensor.matmul(
                                    pc[:], lv[:, j], rv[:, j],
                                    start=(i == 0), stop=(i == n_dr - 1),
                                    perf_mode=mybir.MatmulPerfMode.DoubleRow,
                                )
                                i += 1
                        o_sb = opool.tile([P, TN], F32)
                        nc.vector.tensor_scalar_mul(o_sb[:], pc[:], 2.0 ** -16)
                        nc.vector.tensor_tensor(
                            o_sb[:], o_sb[:], pm[:], mybir.AluOpType.add)
                        nc.vector.tensor_tensor(
                            o_sb[:], o_sb[:], bias_sb[:, n * TN:(n + 1) * TN],
                            mybir.AluOpType.add)
                        nc.sync.dma_start(
                            out[m * TM:(m + 1) * TM, n * TN:(n + 1) * TN], o_sb[:]
                        )
    nc.compile()
    return nc


def get_nc():
    if "nc" not in _cache:
        _cache["nc"] = _build_nc()
    return _cache["nc"]


def _rtn12(x):
    """Round fp32 to 12 significand bits (float32r passes these through
    bit-exactly)."""
    _, e = np.frexp(x.astype(np.float64))
    scale = np.ldexp(1.0, e - 12)
    with np.errstate(invalid="ignore", divide="ignore"):
        r = np.rint(x.astype(np.float64) / scale) * scale
    return np.where(x == 0.0, 0.0, r).astype(np.float32)


def _blk_x(a2d, dt):
    """[M, K] -> [MT, P, KT, TM] with blk[m, p, k, j] = a2d[m*TM+j, k*P+p]."""
    aT = np.ascontiguousarray(a2d.T)  # [K, M]
    return np.ascontiguousarray(
        aT.reshape(KT, P, MT, TM).transpose(2, 1, 0, 3)).astype(dt)


def _blk_w(wt, dt):
    """[K, N] -> [NT, P, KT, TN] with blk[n, p, k, j] = wt[k*P+p, n*TN+j]."""
    return np.ascontiguousarray(
        wt.reshape(KT, P, NT, TN).transpose(2, 1, 0, 3)).astype(dt)


def prep_in_maps(x, W, b):
    x = np.asarray(x, dtype=np.float32)
    W = np.asarray(W, dtype=np.float32)
    b = np.asarray(b, dtype=np.float32)

    A = x.reshape(SEQ * BATCH, K)
    wh12 = _rtn12(W)
    dw = (W.astype(np.float64) - wh12) * (2.0 ** 16)
    whb = _blk_w(np.ascontiguousarray(wh12.T), np.float32)
    w8b = _blk_w(np.ascontiguousarray(W.T * 16.0), E4M3)
    dwb = _blk_w(np.ascontiguousarray(dw.T.astype(np.float32)), E4M3)
    bias_bcast = np.ascontiguousarray(np.broadcast_to(b, (P, N)))

    in_maps = []
    for c in range(N_CORES):
        Ac = A[c * M:(c + 1) * M]
        ah12 = _rtn12(Ac)
        dxs = (Ac.astype(np.float64) - ah12) * (2.0 ** 12)
        in_maps.append({
            "xh": _blk_x(ah12, np.float32),
            "dx8": _blk_x(dxs.astype(np.float32), E4M3),
            "x8": _blk_x(Ac, E4M3),
            "wh": whb, "w8": w8b, "dw8": dwb, "bias": bias_bcast,
        })
    return in_maps


def kernel(x, W, b):
    from concourse.bass_utils import run_bass_kernel_spmd

    in_maps = prep_in_maps(x, W, b)
    nc = get_nc()
    res = run_bass_kernel_spmd(nc, in_maps, core_ids=list(range(N_CORES)))
    full = np.concatenate([r["out"] for r in res.results], axis=0)
    return full.reshape(SEQ, BATCH, OUT_DIM).astype(np.float32)


# revision 5
# speedup vs baseline: 1.6236x; 1.1516x over previous
"""Trainium2 Bass kernel for nn_BOREP (dense_mlp):

    out[s, b, o] = einsum('sbi,oi->sbo', x, W) + bias[o]
    x [256, 64, 1024] f32, W [4096, 1024] f32, bias [4096] f32 -> out [256, 64, 4096] f32

Strategy
--------
Data-parallel over 8 NeuronCores: shard x along seq (axis 0), 32 timesteps per
core, i.e. per-core A = x-shard reshaped to [2048, 1024]; W and bias
replicated. Per core: out_shard = A @ W.T + bias -> [2048, 4096].

Per-core numeric scheme ("f32r main + fp8-DoubleRow correction"):
TRN2's PE runs fp32 matmul at 4 cycles/row, but the `float32r` dtype streams
at 1 cycle/row (free dim >= 256) while keeping exactly 12 significand bits
(HW-verified: 12-bit values pass through bit-exactly in both operand roles).
So the fp32 product is computed as an exact 12-bit main term plus a small
correction evaluated in fp8 at double rate:

    xh = rtn12(x), dx = x - xh   (|dx| <= 2^-12 |x|);  wh = rtn12(W), dw likewise
    A @ W.T  =  Ah @ Wh.T                   exact products of 12-bit values,
                                            float32r @ 1 cyc/row
             +  (dx @ W.T + A @ dw.T)       ~2^-12-scale correction, e4m3 fp8
                                            with DoubleRow perf mode (2 k-tiles
                                            per instruction, ~0.5 cyc/row)
             (+ dx @ dw.T ~ 2^-24, dropped)

The fp8 correction operands carry power-of-2 scales chosen so both cross
products land in one PSUM bank at a common 2^16 scale: dx8 = e4m3(dx * 2^12),
w8 = e4m3(W * 2^4), x8 = e4m3(x), dw8 = e4m3(dw * 2^16). The final output is
out = psum_main + 2^-16 * psum_cross + bias (DVE ops during PSUM->SBUF copy).
Total PE cost ~2.1 cycles/row vs 4 for native fp32. HW-measured: ~345 us/core
body (vs ~1100 us native fp32, ~630 us for an fp16 triple-split), max rel
error 4.3e-06 (absmax ~5.9e-05 on an output scale of 13.6), bit-deterministic.

Layout: host pre-blocks operands so every DMA lands [128, kt, free] tiles with
>=1KB-contiguous runs per partition; contraction dim k on SBUF partitions.
Loop is n-outer with the x-side SBUF-resident (~96KB/partition) and W streamed
once (24MB total traffic), double-buffered; each [128m, 512n] output tile uses
two PSUM banks (main + cross), 4-deep pipelining.
"""
import sys

if "/opt/trn_rl_repo" not in sys.path:
    sys.path.insert(0, "/opt/trn_rl_repo")

import numpy as np
import ml_dtypes

# Problem constants (hardcoded per contest contract)
SEQ, BATCH, IN_DIM, OUT_DIM = 256, 64, 1024, 4096
N_CORES = 8
P = 128
K = IN_DIM
M = SEQ * BATCH // N_CORES     # 2048 rows per core
N = OUT_DIM
KT = K // P                    # 8 k-tiles
TM = 128                       # out-tile rows (PSUM partitions)
TN = 512                       # out-tile cols (one PSUM bank of fp32)
MT = M // TM                   # 16
NT = N // TN                   # 8

E4M3 = ml_dtypes.float8_e4m3

_cache = {}


def _build_nc(repeat: int = 1):
    import concourse.mybir as mybir
    import concourse.tile as tile
    from concourse import bacc
    from contextlib import ExitStack

    F32 = mybir.dt.float32
    F32R = mybir.dt.float32r
    F8 = mybir.dt.float8e4

    nc = bacc.Bacc("TRN2", target_bir_lowering=False, debug=False)

    xh = nc.dram_tensor("xh", [MT, P, KT, TM], F32R, kind="ExternalInput").ap()
    dx8 = nc.dram_tensor("dx8", [MT, P, KT, TM], F8, kind="ExternalInput").ap()
    x8 = nc.dram_tensor("x8", [MT, P, KT, TM], F8, kind="ExternalInput").ap()
    wh = nc.dram_tensor("wh", [NT, P, KT, TN], F32R, kind="ExternalInput").ap()
    w8 = nc.dram_tensor("w8", [NT, P, KT, TN], F8, kind="ExternalInput").ap()
    dw8 = nc.dram_tensor("dw8", [NT, P, KT, TN], F8, kind="ExternalInput").ap()
    bias = nc.dram_tensor("bias", [P, N], F32, kind="ExternalInput").ap()
    out = nc.dram_tensor("out", [M, N], F32, kind="ExternalOutput").ap()

    with tile.TileContext(nc) as tc:
        with ExitStack() as ctx:
            xpool = ctx.enter_context(tc.tile_pool(name="xpool", bufs=1))
            wpool = ctx.enter_context(tc.tile_pool(name="wpool", bufs=2))
            opool = ctx.enter_context(tc.tile_pool(name="opool", bufs=6))
            cpool = ctx.enter_context(tc.tile_pool(name="cpool", bufs=1))
            ps = ctx.enter_context(tc.tile_pool(name="ps", bufs=4, space="PSUM"))

            bias_sb = cpool.tile([P, N], F32)

            for _ in range(repeat):
                # DMA emission order = consumption order: x m=0 slices, then
                # the W n=0 slices (the PE's first operands), then bias (first
                # DVE use a few us in), then the rest of x. W n>=1 is emitted
                # inside the n-loop and prefetches one slice ahead (bufs=2).
                xh_sb, dx_sb, x8_sb = [], [], []

                def load_x(m):
                    t1 = xpool.tile([P, KT, TM], F32R, tag=f"xh_{m}")
                    nc.sync.dma_start(t1[:], xh[m])
                    t2 = xpool.tile([P, KT, TM], F8, tag=f"dx_{m}")
                    nc.sync.dma_start(t2[:], dx8[m])
                    t3 = xpool.tile([P, KT, TM], F8, tag=f"x8_{m}")
                    nc.sync.dma_start(t3[:], x8[m])
                    xh_sb.append(t1); dx_sb.append(t2); x8_sb.append(t3)

                load_x(0)
                w0h = wpool.tile([P, KT, TN], F32R, tag="wh")
                nc.sync.dma_start(w0h[:], wh[0])
                w08 = wpool.tile([P, KT, TN], F8, tag="w8")
                nc.sync.dma_start(w08[:], w8[0])
                w0d = wpool.tile([P, KT, TN], F8, tag="dw")
                nc.sync.dma_start(w0d[:], dw8[0])
                nc.sync.dma_start(bias_sb[:], bias[:])
                for m in range(1, MT):
                    load_x(m)

                for n in range(NT):
                    if n == 0:
                        wh_sb, w8_sb, dw_sb = w0h, w08, w0d
                    else:
                        wh_sb = wpool.tile([P, KT, TN], F32R, tag="wh")
                        nc.sync.dma_start(wh_sb[:], wh[n])
                        w8_sb = wpool.tile([P, KT, TN], F8, tag="w8")
                        nc.sync.dma_start(w8_sb[:], w8[n])
                        dw_sb = wpool.tile([P, KT, TN], F8, tag="dw")
                        nc.sync.dma_start(dw_sb[:], dw8[n])

                    for m in range(MT):
                        pm = ps.tile([P, TN], F32)
                        for k in range(KT):
                            nc.tensor.matmul(
                                pm[:], xh_sb[m][:, k], wh_sb[:, k],
                                start=(k == 0), stop=(k == KT - 1),
                            )
                        pc = ps.tile([P, TN], F32)
                        # DoubleRow: [P, KT, X] viewed as [P, KT//2, 2, X];
                        # each instruction contracts 2 k-tiles (256 values).
                        dxv = dx_sb[m].rearrange("p (j i) t -> p j i t", i=2)
                        x8v = x8_sb[m].rearrange("p (j i) t -> p j i t", i=2)
                        w8v = w8_sb.rearrange("p (j i) t -> p j i t", i=2)
                        dwv = dw_sb.rearrange("p (j i) t -> p j i t", i=2)
                        n_dr = KT
                        i = 0
                        for (lv, rv) in ((dxv, w8v), (x8v, dwv)):
                            for j in range(KT // 2):
                                nc.tensor.matmul(
                                    pc[:], lv[:, j], rv[:, j],
                                    start=(i == 0), stop=(i == n_dr - 1),
                                    perf_mode=mybir.MatmulPerfMode.DoubleRow,
                                )
                                i += 1
                        o_sb = opool.tile([P, TN], F32)
                        nc.vector.tensor_scalar_mul(o_sb[:], pc[:], 2.0 ** -16)
                        nc.vector.tensor_tensor(
                            o_sb[:], o_sb[:], pm[:], mybir.AluOpType.add)
                        nc.vector.tensor_tensor(
                            o_sb[:], o_sb[:], bias_sb[:, n * TN:(n + 1) * TN],
                            mybir.AluOpType.add)
                        nc.sync.dma_start(
                            out[m * TM:(m + 1) * TM, n * TN:(n + 1) * TN], o_sb[:]
                        )
    nc.compile()
    return nc


def get_nc():
    if "nc" not in _cache:
        _cache["nc"] = _build_nc()
    return _cache["nc"]


def _rtn12(x):
    """Round fp32 to 12 significand bits (float32r passes these through
    bit-exactly)."""
    _, e = np.frexp(x.astype(np.float64))
    scale = np.ldexp(1.0, e - 12)
    with np.errstate(invalid="ignore", divide="ignore"):
        r = np.rint(x.astype(np.float64) / scale) * scale
    return np.where(x == 0.0, 0.0, r).astype(np.float32)


def _blk_x(a2d, dt):
    """[M, K] -> [MT, P, KT, TM] with blk[m, p, k, j] = a2d[m*TM+j, k*P+p]."""
    aT = np.ascontiguousarray(a2d.T)  # [K, M]
    return np.ascontiguousarray(
        aT.reshape(KT, P, MT, TM).transpose(2, 1, 0, 3)).astype(dt)


def _blk_w(wt, dt):
    """[K, N] -> [NT, P, KT, TN] with blk[n, p, k, j] = wt[k*P+p, n*TN+j]."""
    return np.ascontiguousarray(
        wt.reshape(KT, P, NT, TN).transpose(2, 1, 0, 3)).astype(dt)


def prep_in_maps(x, W, b):
    x = np.asarray(x, dtype=np.float32)
    W = np.asarray(W, dtype=np.float32)
    b = np.asarray(b, dtype=np.float32)

    A = x.reshape(SEQ * BATCH, K)
    wh12 = _rtn12(W)
    dw = (W.astype(np.float64) - wh12) * (2.0 ** 16)
    whb = _blk_w(np.ascontiguousarray(wh12.T), np.float32)
    w8b = _blk_w(np.ascontiguousarray(W.T * 16.0), E4M3)
    dwb = _blk_w(np.ascontiguousarray(dw.T.astype(np.float32)), E4M3)
    bias_bcast = np.ascontiguousarray(np.broadcast_to(b, (P, N)))

    in_maps = []
    for c in range(N_CORES):
        Ac = A[c * M:(c + 1) * M]
        ah12 = _rtn12(Ac)
        dxs = (Ac.astype(np.float64) - ah12) * (2.0 ** 12)
        in_maps.append({
            "xh": _blk_x(ah12, np.float32),
            "dx8": _blk_x(dxs.astype(np.float32), E4M3),
            "x8": _blk_x(Ac, E4M3),
            "wh": whb, "w8": w8b, "dw8": dwb, "bias": bias_bcast,
        })
    return in_maps


def kernel(x, W, b):
    from concourse.bass_utils import run_bass_kernel_spmd

    in_maps = prep_in_maps(x, W, b)
    nc = get_nc()
    res = run_bass_kernel_spmd(nc, in_maps, core_ids=list(range(N_CORES)))
    full = np.concatenate([r["out"] for r in res.results], axis=0)
    return full.reshape(SEQ, BATCH, OUT_DIM).astype(np.float32)


# revision 6
# speedup vs baseline: 1.6744x; 1.0313x over previous
"""Trainium2 Bass kernel for nn_BOREP (dense_mlp):

    out[s, b, o] = einsum('sbi,oi->sbo', x, W) + bias[o]
    x [256, 64, 1024] f32, W [4096, 1024] f32, bias [4096] f32 -> out [256, 64, 4096] f32

Strategy
--------
Data-parallel over 8 NeuronCores: shard x along seq (axis 0), 32 timesteps per
core, i.e. per-core A = x-shard reshaped to [2048, 1024]; W and bias
replicated. Per core: out_shard = A @ W.T + bias -> [2048, 4096].

Per-core numeric scheme ("f32r main + fp8-DoubleRow correction"):
TRN2's PE runs fp32 matmul at 4 cycles/row, but the `float32r` dtype streams
at 1 cycle/row (free dim >= 256) while keeping exactly 12 significand bits
(HW-verified: 12-bit values pass through bit-exactly in both operand roles).
So the fp32 product is computed as an exact 12-bit main term plus a small
correction evaluated in fp8 at double rate:

    xh = rtn12(x), dx = x - xh   (|dx| <= 2^-12 |x|);  wh = rtn12(W), dw likewise
    A @ W.T  =  Ah @ Wh.T                   exact products of 12-bit values,
                                            float32r @ 1 cyc/row
             +  (dx @ W.T + A @ dw.T)       ~2^-12-scale correction, e4m3 fp8
                                            with DoubleRow perf mode (2 k-tiles
                                            per instruction, ~0.5 cyc/row)
             (+ dx @ dw.T ~ 2^-24, dropped)

The fp8 correction operands carry power-of-2 scales chosen so both cross
products land in one PSUM bank at a common 2^16 scale: dx8 = e4m3(dx * 2^12),
w8 = e4m3(W * 2^4), x8 = e4m3(x), dw8 = e4m3(dw * 2^16). The final output is
out = psum_main + 2^-16 * psum_cross + bias (DVE ops during PSUM->SBUF copy).
Total PE cost ~2.1 cycles/row vs 4 for native fp32. HW-measured: ~345 us/core
body (vs ~1100 us native fp32, ~630 us for an fp16 triple-split), max rel
error 4.3e-06 (absmax ~5.9e-05 on an output scale of 13.6), bit-deterministic.

Layout: host pre-blocks operands so every DMA lands [128, kt, free] tiles with
>=1KB-contiguous runs per partition; contraction dim k on SBUF partitions.
Loop is n-outer with the x-side SBUF-resident (~96KB/partition) and W streamed
once (24MB total traffic), double-buffered; each [128m, 512n] output tile uses
two PSUM banks (main + cross), 4-deep pipelining.
"""
import sys

if "/opt/trn_rl_repo" not in sys.path:
    sys.path.insert(0, "/opt/trn_rl_repo")

import numpy as np
import ml_dtypes

# Problem constants (hardcoded per contest contract)
SEQ, BATCH, IN_DIM, OUT_DIM = 256, 64, 1024, 4096
N_CORES = 8
P = 128
K = IN_DIM
M = SEQ * BATCH // N_CORES     # 2048 rows per core
N = OUT_DIM
KT = K // P                    # 8 k-tiles
TM = 128                       # out-tile rows (PSUM partitions)
TN = 512                       # out-tile cols (one PSUM bank of fp32)
MT = M // TM                   # 16
NT = N // TN                   # 8

E4M3 = ml_dtypes.float8_e4m3

_cache = {}


def _build_nc(repeat: int = 1):
    import concourse.mybir as mybir
    import concourse.tile as tile
    from concourse import bacc
    from contextlib import ExitStack

    F32 = mybir.dt.float32
    F32R = mybir.dt.float32r
    F8 = mybir.dt.float8e4

    nc = bacc.Bacc("TRN2", target_bir_lowering=False, debug=False)

    xh = nc.dram_tensor("xh", [MT, P, KT, TM], F32R, kind="ExternalInput").ap()
    dx8 = nc.dram_tensor("dx8", [MT, P, KT, TM], F8, kind="ExternalInput").ap()
    x8 = nc.dram_tensor("x8", [MT, P, KT, TM], F8, kind="ExternalInput").ap()
    wh = nc.dram_tensor("wh", [NT, P, KT, TN], F32R, kind="ExternalInput").ap()
    w8 = nc.dram_tensor("w8", [NT, P, KT, TN], F8, kind="ExternalInput").ap()
    dw8 = nc.dram_tensor("dw8", [NT, P, KT, TN], F8, kind="ExternalInput").ap()
    bias = nc.dram_tensor("bias", [P, N], F32, kind="ExternalInput").ap()
    out = nc.dram_tensor("out", [M, N], F32, kind="ExternalOutput").ap()

    with tile.TileContext(nc) as tc:
        with ExitStack() as ctx:
            xpool = ctx.enter_context(tc.tile_pool(name="xpool", bufs=1))
            wpool = ctx.enter_context(tc.tile_pool(name="wpool", bufs=2))
            opool = ctx.enter_context(tc.tile_pool(name="opool", bufs=6))
            cpool = ctx.enter_context(tc.tile_pool(name="cpool", bufs=1))
            ps = ctx.enter_context(tc.tile_pool(name="ps", bufs=4, space="PSUM"))

            bias_sb = cpool.tile([P, N], F32)

            for _ in range(repeat):
                # DMA emission order = consumption order: x m=0 slices, then
                # the W n=0 slices (the PE's first operands), then bias (first
                # DVE use a few us in), then the rest of x. W n>=1 is emitted
                # inside the n-loop and prefetches one slice ahead (bufs=2).
                xh_sb, dx_sb, x8_sb = [], [], []

                def load_x(m):
                    t1 = xpool.tile([P, KT, TM], F32R, tag=f"xh_{m}")
                    nc.sync.dma_start(t1[:], xh[m])
                    t2 = xpool.tile([P, KT, TM], F8, tag=f"dx_{m}")
                    nc.sync.dma_start(t2[:], dx8[m])
                    t3 = xpool.tile([P, KT, TM], F8, tag=f"x8_{m}")
                    nc.sync.dma_start(t3[:], x8[m])
                    xh_sb.append(t1); dx_sb.append(t2); x8_sb.append(t3)

                # First operands in fine grain: xh[0] whole, wh[0] per k-tile
                # (first matmul starts after one 256KB chunk), then the fp8
                # correction operands; bias rides behind x[3] (first DVE use
                # is much later than the PE's first x needs).
                t1 = xpool.tile([P, KT, TM], F32R, tag="xh_0")
                nc.sync.dma_start(t1[:], xh[0])
                xh_sb.append(t1)
                w0h = wpool.tile([P, KT, TN], F32R, tag="wh")
                for k in range(KT):
                    nc.sync.dma_start(w0h[:, k], wh[0, :, k])
                t2 = xpool.tile([P, KT, TM], F8, tag="dx_0")
                nc.sync.dma_start(t2[:], dx8[0])
                t3 = xpool.tile([P, KT, TM], F8, tag="x8_0")
                nc.sync.dma_start(t3[:], x8[0])
                dx_sb.append(t2); x8_sb.append(t3)
                w08 = wpool.tile([P, KT, TN], F8, tag="w8")
                nc.sync.dma_start(w08[:], w8[0])
                w0d = wpool.tile([P, KT, TN], F8, tag="dw")
                nc.sync.dma_start(w0d[:], dw8[0])
                for m in range(1, MT):
                    load_x(m)
                    if m == 3:
                        nc.sync.dma_start(bias_sb[:], bias[:])

                for n in range(NT):
                    if n == 0:
                        wh_sb, w8_sb, dw_sb = w0h, w08, w0d
                    else:
                        wh_sb = wpool.tile([P, KT, TN], F32R, tag="wh")
                        nc.sync.dma_start(wh_sb[:], wh[n])
                        w8_sb = wpool.tile([P, KT, TN], F8, tag="w8")
                        nc.sync.dma_start(w8_sb[:], w8[n])
                        dw_sb = wpool.tile([P, KT, TN], F8, tag="dw")
                        nc.sync.dma_start(dw_sb[:], dw8[n])

                    for m in range(MT):
                        pm = ps.tile([P, TN], F32)
                        for k in range(KT):
                            nc.tensor.matmul(
                                pm[:], xh_sb[m][:, k], wh_sb[:, k],
                                start=(k == 0), stop=(k == KT - 1),
                            )
                        pc = ps.tile([P, TN], F32)
                        # DoubleRow: [P, KT, X] viewed as [P, KT//2, 2, X];
                        # each instruction contracts 2 k-tiles (256 values).
                        dxv = dx_sb[m].rearrange("p (j i) t -> p j i t", i=2)
                        x8v = x8_sb[m].rearrange("p (j i) t -> p j i t", i=2)
                        w8v = w8_sb.rearrange("p (j i) t -> p j i t", i=2)
                        dwv = dw_sb.rearrange("p (j i) t -> p j i t", i=2)
                        n_dr = KT
                        i = 0
                        for (lv, rv) in ((dxv, w8v), (x8v, dwv)):
                            for j in range(KT // 2):
                                nc.tensor.matmul(
                                    pc[:], lv[:, j], rv[:, j],
                                    start=(i == 0), stop=(i == n_dr - 1),
                                    perf_mode=mybir.MatmulPerfMode.DoubleRow,
                                )
                                i += 1
                        o_sb = opool.tile([P, TN], F32)
                        nc.vector.tensor_scalar_mul(o_sb[:], pc[:], 2.0 ** -16)
                        nc.vector.tensor_tensor(
                            o_sb[:], o_sb[:], pm[:], mybir.AluOpType.add)
                        nc.vector.tensor_tensor(
                            o_sb[:], o_sb[:], bias_sb[:, n * TN:(n + 1) * TN],
                            mybir.AluOpType.add)
                        nc.sync.dma_start(
                            out[m * TM:(m + 1) * TM, n * TN:(n + 1) * TN], o_sb[:]
                        )
    nc.compile()
    return nc


def get_nc():
    if "nc" not in _cache:
        _cache["nc"] = _build_nc()
    return _cache["nc"]


def _rtn12(x):
    """Round fp32 to 12 significand bits (float32r passes these through
    bit-exactly)."""
    _, e = np.frexp(x.astype(np.float64))
    scale = np.ldexp(1.0, e - 12)
    with np.errstate(invalid="ignore", divide="ignore"):
        r = np.rint(x.astype(np.float64) / scale) * scale
    return np.where(x == 0.0, 0.0, r).astype(np.float32)


def _blk_x(a2d, dt):
    """[M, K] -> [MT, P, KT, TM] with blk[m, p, k, j] = a2d[m*TM+j, k*P+p]."""
    aT = np.ascontiguousarray(a2d.T)  # [K, M]
    return np.ascontiguousarray(
        aT.reshape(KT, P, MT, TM).transpose(2, 1, 0, 3)).astype(dt)


def _blk_w(wt, dt):
    """[K, N] -> [NT, P, KT, TN] with blk[n, p, k, j] = wt[k*P+p, n*TN+j]."""
    return np.ascontiguousarray(
        wt.reshape(KT, P, NT, TN).transpose(2, 1, 0, 3)).astype(dt)


def prep_in_maps(x, W, b):
    x = np.asarray(x, dtype=np.float32)
    W = np.asarray(W, dtype=np.float32)
    b = np.asarray(b, dtype=np.float32)

    A = x.reshape(SEQ * BATCH, K)
    wh12 = _rtn12(W)
    dw = (W.astype(np.float64) - wh12) * (2.0 ** 16)
    whb = _blk_w(np.ascontiguousarray(wh12.T), np.float32)
    w8b = _blk_w(np.ascontiguousarray(W.T * 16.0), E4M3)
    dwb = _blk_w(np.ascontiguousarray(dw.T.astype(np.float32)), E4M3)
    bias_bcast = np.ascontiguousarray(np.broadcast_to(b, (P, N)))

    in_maps = []
    for c in range(N_CORES):
        Ac = A[c * M:(c + 1) * M]
        ah12 = _rtn12(Ac)
        dxs = (Ac.astype(np.float64) - ah12) * (2.0 ** 12)
        in_maps.append({
            "xh": _blk_x(ah12, np.float32),
            "dx8": _blk_x(dxs.astype(np.float32), E4M3),
            "x8": _blk_x(Ac, E4M3),
            "wh": whb, "w8": w8b, "dw8": dwb, "bias": bias_bcast,
        })
    return in_maps


def kernel(x, W, b):
    from concourse.bass_utils import run_bass_kernel_spmd

    in_maps = prep_in_maps(x, W, b)
    nc = get_nc()
    res = run_bass_kernel_spmd(nc, in_maps, core_ids=list(range(N_CORES)))
    full = np.concatenate([r["out"] for r in res.results], axis=0)
    return full.reshape(SEQ, BATCH, OUT_DIM).astype(np.float32)
